# revision 34
# baseline (speedup 1.0000x reference)
"""Causal multi-head attention (B=2, L=2048, D=1024, H=16, Dh=64) on 8 TRN2
NeuronCores.

Sharding: data-parallel over B (2 groups of 4 cores), tensor-parallel over H
within a group (4 heads per core). Each core computes QKV projections for its
heads, full causal attention per head (flash-style, scores kept transposed so
no on-chip transposes are needed), and a partial output projection
y_c = sum_h o_h @ Wout_h. The host sums the 4 partials per batch.

Per-core layout choices:
  - x is pre-transposed on the host (xT [D, L]) so the QKV contraction dim D
    lands on SBUF partitions directly.
  - q, k are produced transposed (qT/kT [e, L]) so the scores matmul
    ST = K_h @ Q_h^T contracts over Dh on partitions; softmax runs on ST
    tiles [k=128, q=512] with the reduction (sum over k) folded into the
    P@V matmul via a ones-row appended to V (lhsT [128, 65]; row 64 of the
    PSUM result is the softmax denominator).
  - Scores/exp/P@V skip columns left of the causal diagonal block.
  - Emission priority: attention steps for a q-chunk are emitted BEFORE the
    filler work (next chunk's QKV, norm-backs, projections) so the Tile
    scheduler always prefers feeding the scalar engine's exp stream and
    fills PE stalls with the later-priority matmuls.
"""

import numpy as np

import concourse.bass as bass
import concourse.mybir as mybir
import concourse.tile as tile
from concourse import bacc
from concourse.bass import broadcast_tensor_aps
from concourse.bass_utils import run_bass_kernel_spmd

F32 = mybir.dt.float32
BF16 = mybir.dt.bfloat16
EXP = mybir.ActivationFunctionType.Exp
MULT = mybir.AluOpType.mult

B, L, D, H = 2, 2048, 1024, 16
Dh = D // H
NCORES = 8
NH = 4            # heads per core
EL = NH * Dh      # local head dims = 256
P = 128
NQ = 512          # q-chunk width (scores free dim)
QC = L // NQ      # 4 q-chunks
DC = D // P       # 8 contraction chunks for projections
LC = 4            # xT l-chunks for QKV
NL = L // LC      # 512


def build():
    nc = bacc.Bacc("TRN2", target_bir_lowering=False, debug=False,
                   num_devices=NCORES)

    xT = nc.dram_tensor("xT", [D, L], BF16, kind="ExternalInput")
    wq = nc.dram_tensor("wq", [D, EL], BF16, kind="ExternalInput")
    wk = nc.dram_tensor("wk", [D, EL], BF16, kind="ExternalInput")
    wv = nc.dram_tensor("wv", [D, EL], BF16, kind="ExternalInput")
    wout = nc.dram_tensor("wout", [EL, D], BF16, kind="ExternalInput")
    masks = nc.dram_tensor("masks", [P, P], BF16, kind="ExternalInput")
    out = nc.dram_tensor("out", [L, D], F32, kind="ExternalOutput")

    scale = 1.0 / np.sqrt(Dh)

    with tile.TileContext(nc) as tc:
        with (
            tc.tile_pool(name="const", bufs=1) as cpool,
            tc.tile_pool(name="xt", bufs=2) as xpool,
            tc.tile_pool(name="pt", bufs=6) as ptpool,
            tc.tile_pool(name="work", bufs=3) as wpool,
            tc.tile_pool(name="norm", bufs=8) as npool,
            tc.tile_pool(name="dram", bufs=8, space="DRAM") as dpool,
            tc.tile_pool(name="mm", bufs=2, space="PSUM") as mm_ps,
            tc.tile_pool(name="st", bufs=2, space="PSUM") as st_ps,
            tc.tile_pool(name="pv", bufs=2, space="PSUM") as pv_ps,
        ):
            # ---- persistent SBUF tensors ----
            wq_sb = cpool.tile([P, DC, EL], BF16, tag="wq")
            wk_sb = cpool.tile([P, DC, EL], BF16, tag="wk")
            wv_sb = cpool.tile([P, DC, EL], BF16, tag="wv")
            wout_sb = cpool.tile([P, EL // P, D], BF16, tag="wout")
            mask_sb = cpool.tile([P, P], BF16, tag="mask")
            qT_sb = cpool.tile([P, EL // P, L], BF16, tag="qT")
            kT_sb = cpool.tile([P, EL // P, L], BF16, tag="kT")
            vext_sb = cpool.tile([P, L // P, NH, Dh + 1], BF16, tag="vext")
            oT_sb = cpool.tile([P, EL // P, L], BF16, tag="oT")
            ones_f32 = cpool.tile([P, P], F32, tag="onesf")
            junk_sb = cpool.tile([P, NQ], BF16, tag="junk")

            # DMA order = need order, with few large dispatches (each
            # dma_start costs ~0.3-0.6us of sequencer dispatch): the first
            # qk matmul group consumes (wq chunk dc, xt0 chunk dc) in dc
            # order, so interleave halves of those first; spread dispatch
            # across the two HWDGE sequencers (SP + ACT, which is idle
            # during the prologue).
            xT_r = xT.ap().rearrange("(o p) l -> p o l", p=P)
            wq_r = wq.ap().rearrange("(o p) e -> p o e", p=P)
            xt0 = xpool.tile([P, DC, NL], BF16, tag="xt", name="xt0")
            hd = DC // 2
            nc.sync.dma_start(wq_sb[:, 0:hd, :], wq_r[:, 0:hd, :])
            nc.scalar.dma_start(xt0[:, 0:hd, :], xT_r[:, 0:hd, 0:NL])
            nc.sync.dma_start(wq_sb[:, hd:, :], wq_r[:, hd:, :])
            nc.scalar.dma_start(xt0[:, hd:, :], xT_r[:, hd:, 0:NL])
            nc.sync.dma_start(mask_sb[:], masks[:, :])
            nc.sync.dma_start(
                wk_sb[:], wk.ap().rearrange("(o p) e -> p o e", p=P))
            nc.scalar.dma_start(
                wv_sb[:], wv.ap().rearrange("(o p) e -> p o e", p=P))
            nc.sync.dma_start(
                wout_sb[:], wout.ap().rearrange("(o p) d -> p o d", p=P))

            nc.vector.memset(junk_sb[:], 0.25)
            nc.vector.memset(ones_f32[:], 1.0)
            # ones column of vext (the softmax-denominator row of P@V)
            nc.vector.tensor_copy(
                out=vext_sb[:, :, :, Dh],
                in_=ones_f32[:, 0:L // P * NH].rearrange("p (a b) -> p a b", a=L // P),
            )
            # preload the exp table set during the DMA wait so the first
            # real exp doesn't pay the ~2.7us table load
            junk2 = wpool.tile([P, Dh], BF16, tag="junk2")
            nc.scalar.activation(out=junk2[:], in_=junk_sb[:, 0:Dh],
                                 func=EXP, scale=1.0)
            # HAM warm-up: dummy matmuls keep the PE busy while the first
            # weight/activation DMAs land, so real work starts at 2.4 GHz
            for i in range(8):
                ps = mm_ps.tile([P, NQ], F32, tag="mm", name=f"warm_{i}")
                nc.tensor.matmul(ps[:], junk_sb[:, 0:P], junk_sb[:],
                                 start=True, stop=True)

            def emit_qkv(lc):
                if lc == 0:
                    xt = xt0
                else:
                    xt = xpool.tile([P, DC, NL], BF16, tag="xt",
                                    name=f"xt{lc}")
                    for dc in range(0, DC, 2):
                        nc.sync.dma_start(
                            xt[:, dc:dc + 2, :],
                            xT_r[:, dc:dc + 2, lc * NL:(lc + 1) * NL])
                # ec-major so attention pair 0 (heads 0/1 = ec 0) unblocks
                # after the first two groups
                gi = 0
                for ec in range(EL // P):
                    for w_sb, dst in ((wq_sb, qT_sb), (wk_sb, kT_sb)):
                        ps = mm_ps.tile([P, NQ], F32, tag="mm",
                                        name=f"qk_{lc}_{ec}")
                        for dc in range(DC):
                            nc.tensor.matmul(
                                ps[:],
                                w_sb[:, dc, ec * P:(ec + 1) * P],
                                xt[:, dc, :],
                                start=(dc == 0), stop=(dc == DC - 1),
                            )
                        nc.vector.tensor_copy(
                            out=dst[:, ec, lc * NL:(lc + 1) * NL], in_=ps[:])
                        if lc == 0:
                            # dependency-free fillers between the first qk
                            # groups: they plug the input-DMA-paced gaps in
                            # the PE stream so the HAM clock gate reaches
                            # 8/8 during the ramp instead of ~20us in
                            for i in range(2):
                                fp = pv_ps.tile([P, NQ], F32, tag="pv",
                                                name=f"rfill_{gi}_{i}")
                                nc.tensor.matmul(
                                    fp[:, 0:NQ // 2], junk_sb[:, 0:P],
                                    junk_sb[:, 0:NQ // 2],
                                    start=True, stop=True)
                            gi += 1

                for lt in range(NL // P):
                    lo = lc * (NL // P) + lt
                    ps = mm_ps.tile([P, EL], F32, tag="mm",
                                    name=f"v_{lc}_{lt}")
                    for dc in range(DC):
                        nc.tensor.matmul(
                            ps[:],
                            xt[:, dc, lt * P:(lt + 1) * P],
                            wv_sb[:, dc, :],
                            start=(dc == 0), stop=(dc == DC - 1),
                        )
                    nc.vector.tensor_copy(
                        out=vext_sb[:, lo, :, 0:Dh],
                        in_=ps[:].rearrange("p (h e) -> p h e", h=NH),
                    )
                    if lc == 0:
                        for i in range(2):
                            fp = pv_ps.tile([P, NQ], F32, tag="pv",
                                            name=f"vfill_{lt}_{i}")
                            nc.tensor.matmul(
                                fp[:, 0:NQ // 2], junk_sb[:, 0:P],
                                junk_sb[:, 0:NQ // 2],
                                start=True, stop=True)

            norm_state = {}        # (qc, h) -> (ot_un, dr2)

            def emit_attn_pair(qc, pair):
                nk = 4 * (qc + 1)          # causal k-chunks of 128
                heads = (2 * pair, 2 * pair + 1)
                pts = {}               # (h, ki) -> pt tile
                pvs = {}               # h -> accumulating PSUM tile

                def emit_pv(ki):
                    # P@V runs one ki behind the scores so the in-order
                    # PE stream never waits on the exp of the current ki
                    cm = max(0, P * (ki - 4 * qc))
                    for h in heads:
                        if ki == 0:
                            pvs[h] = pv_ps.tile([Dh + 1, NQ], F32,
                                                name=f"po_{qc}_{h}",
                                                tag="pv")
                        nc.tensor.matmul(
                            pvs[h][:, cm:],
                            vext_sb[:, ki, h, :],
                            pts.pop((h, ki))[:, cm:],
                            start=(ki == 0), stop=(ki == nk - 1),
                        )

                # scores are emitted in runs of two ki so consecutive score
                # matmuls alternate PE row groups (heads at partitions 0-63
                # / 64-127) and every stationary load after the first hides
                # under the previous matmul's streaming
                for kb in range(0, nk, 2):
                    kis = [ki for ki in (kb, kb + 1) if ki < nk]
                    stps = {}
                    for ki in kis:
                        j = ki - 4 * qc    # >=0 on diagonal-crossing tiles
                        c0 = max(0, P * j)
                        # both heads' score tiles share one 2-bank PSUM
                        # tile so a single EXP covers the pair
                        stp = st_ps.tile([P, 2, NQ], F32, tag="st",
                                         name=f"st_{qc}_{pair}_{ki}")
                        ptp = ptpool.tile([P, 2, NQ], BF16, tag="pt",
                                          name=f"pt_{qc}_{pair}_{ki}")
                        stps[ki] = (stp, ptp, c0, j)
                        for idx, h in enumerate(heads):
                            hp = (h % 2) * 64
                            ec = h // 2
                            nc.tensor.matmul(
                                stp[:, idx, c0:],
                                kT_sb[hp:hp + 64, ec, ki * P:(ki + 1) * P],
                                qT_sb[hp:hp + 64, ec,
                                      qc * NQ + c0:(qc + 1) * NQ],
                                start=True, stop=True,
                            )
                            pts[(h, ki)] = ptp[:, idx, :]
                    for ki in kis:
                        stp, ptp, c0, j = stps[ki]
                        nc.scalar.activation(
                            out=ptp[:, :, c0:], in_=stp[:, :, c0:],
                            func=EXP, scale=scale)
                        if j >= 0:
                            # the diagonal 128-block needs the triangular
                            # mask
                            for idx in range(2):
                                nc.gpsimd.tensor_tensor(
                                    out=ptp[:, idx, c0:c0 + P],
                                    in0=ptp[:, idx, c0:c0 + P],
                                    in1=mask_sb[:, :], op=MULT)
                    # P@V runs one score-group behind so the in-order PE
                    # stream never waits on a fresh exp
                    for ki in (kb - 2, kb - 1):
                        if ki >= 0:
                            emit_pv(ki)
                for ki in (nk - 2, nk - 1):
                    emit_pv(ki)

                # the last q-chunk's chains are the kernel's tail: route
                # their DMA hops through the ACT HWDGE queue (idle by then)
                # so they don't serialize behind the y-output DMA dispatches
                # on the SP sequencer
                deng = nc.scalar if qc == QC - 1 else nc.sync
                for h in heads:
                    po = pvs[h]
                    # norm front half: evacuate PSUM (pinned to DVE so it
                    # never steals exp throughput from the scalar engine)
                    # and kick off the reciprocal chain
                    ot_un = npool.tile([64, NQ], BF16, tag="otun",
                                       name=f"otun_{qc}_{h}")
                    nc.vector.tensor_copy(out=ot_un[:], in_=po[0:64, :])
                    rsum = npool.tile([P, NQ], F32, tag="rsum",
                                      name=f"rsum_{qc}_{h}")
                    nc.vector.tensor_copy(out=rsum[64:65, :],
                                          in_=po[64:65, :])
                    # reshape the [1,512] rsum row to [64,8] with one
                    # SBUF->SBUF DMA so the reciprocal uses 64 DVE lanes
                    r64 = npool.tile([64, NQ // 64], F32, tag="r64",
                                     name=f"r64_{qc}_{h}")
                    deng.dma_start(r64[:], rsum[64:65, :])
                    nc.vector.reciprocal(r64[:], r64[:])
                    dr2 = dpool.tile([NQ], F32,
                                     name=f"dr2_{qc}_{h}", tag="dr2")
                    deng.dma_start(
                        dr2[:].rearrange("(a b) -> a b", b=NQ // 64),
                        r64[:])
                    norm_state[(qc, h)] = (ot_un, dr2)

            def emit_norm_back(qc):
                for h in range(NH):
                    emit_norm_back_h(qc, h)

            def emit_norm_back_h(qc, h):
                # back half: replicate the reciprocal row to 64 partitions
                # with a stride-0 broadcast DMA from the DRAM bounce, then
                # one DVE multiply; no PE involvement
                hp = (h % 2) * 64
                ec = h // 2
                deng = nc.scalar if qc == QC - 1 else nc.sync
                ot_un, dr2 = norm_state.pop((qc, h))
                rr_bc = npool.tile([64, NQ], F32, tag="rrbc",
                                   name=f"rrbc_{qc}_{h}")
                d_ap, s_ap = broadcast_tensor_aps(
                    rr_bc[:, :], dr2[:].rearrange("(a b) -> a b", a=1))
                deng.dma_start(d_ap, s_ap)
                # the last q-chunk's two per-pair multiplies are on the
                # critical tail: run them on different engines in parallel
                meng = nc.gpsimd if (qc == QC - 1 and h % 2 == 1) else nc.vector
                if hp == 0:
                    # heads on partitions 0-63 can write oT in place —
                    # no partition shift needed, so skip the DMA hop
                    meng.tensor_tensor(
                        out=oT_sb[0:64, ec, qc * NQ:(qc + 1) * NQ],
                        in0=ot_un[:], in1=rr_bc[:], op=MULT)
                else:
                    tmp = wpool.tile([64, NQ], BF16, tag="tmp")
                    meng.tensor_tensor(out=tmp[:], in0=ot_un[:],
                                       in1=rr_bc[:], op=MULT)
                    deng.dma_start(
                        oT_sb[hp:hp + 64, ec, qc * NQ:(qc + 1) * NQ],
                        tmp[:])

            def emit_proj(qc):
                # y = oT^T @ wout (partial over heads) for this q-chunk's rows
                for lt in range(4 * qc, 4 * (qc + 1)):
                    emit_proj_lt(lt)

            def emit_proj_lt(lt, pool=None, tag="mm", fine=False):
                for do in range(D // NQ):
                    ps = (pool or mm_ps).tile([P, NQ], F32, tag=tag,
                                              name=f"y_{lt}_{do}")
                    for ec in range(EL // P):
                        nc.tensor.matmul(
                            ps[:],
                            oT_sb[:, ec, lt * P:(lt + 1) * P],
                            wout_sb[:, ec, do * NQ:(do + 1) * NQ],
                            start=(ec == 0), stop=(ec == EL // P - 1),
                        )
                    y_sb = wpool.tile([P, NQ], F32, tag="y")
                    if fine:
                        # tail tiles: split the evacuation + output DMA in
                        # half across both engines/queues so the last
                        # tile's drain chain is as short as possible
                        hn = NQ // 2
                        nc.vector.tensor_copy(out=y_sb[:, 0:hn],
                                              in_=ps[:, 0:hn])
                        nc.scalar.copy(out=y_sb[:, hn:], in_=ps[:, hn:])
                        nc.sync.dma_start(
                            out.ap()[lt * P:(lt + 1) * P,
                                     do * NQ:do * NQ + hn],
                            y_sb[:, 0:hn])
                        nc.scalar.dma_start(
                            out.ap()[lt * P:(lt + 1) * P,
                                     do * NQ + hn:(do + 1) * NQ],
                            y_sb[:, hn:])
                    else:
                        nc.any.tensor_copy(out=y_sb[:], in_=ps[:])
                        nc.sync.dma_start(
                            out.ap()[lt * P:(lt + 1) * P,
                                     do * NQ:(do + 1) * NQ],
                            y_sb[:])

            # attention for q-chunk ph is emitted BEFORE the fillers
            # (QKV ph+1, norm-backs, projections) so the scheduler keeps
            # the exp stream fed and uses fillers only to plug PE stalls
            emit_qkv(0)
            for ph in range(QC):
                emit_attn_pair(ph, 0)
                emit_attn_pair(ph, 1)
                if ph + 1 < QC:
                    emit_qkv(ph + 1)
                if ph < 2:
                    # the first two phases are input-DMA paced; keep the
                    # PE (and with it the HAM clock gate) busy across the
                    # xt-chunk waits
                    for i in range(4):
                        fp = pv_ps.tile([P, NQ], F32, tag="pv",
                                        name=f"pfill_{ph}_{i}")
                        nc.tensor.matmul(
                            fp[:, 0:NQ // 2], junk_sb[:, 0:P],
                            junk_sb[:, 0:NQ // 2],
                            start=True, stop=True)
                if ph >= 1:
                    emit_norm_back(ph - 1)
                if ph >= 2:
                    emit_proj(ph - 2)
            # epilogue: last norm back-halves, then the remaining
            # projections (qc=2 rows are ready; qc=3 rows wait on the
            # final norm-backs)
            emit_norm_back(QC - 1)
            emit_proj(QC - 2)
            for i, lt in enumerate(range(12, 16)):
                if i % 2 == 0:
                    emit_proj_lt(lt, pool=st_ps, tag="st", fine=(lt >= 14))
                else:
                    emit_proj_lt(lt, fine=(lt >= 14))
            # lowest-priority dependency-free matmuls: the scheduler only
            # picks these when nothing real is ready, so they plug PE idle
            # gaps (the tail's norm-chain wait) and keep the HAM clock
            # gate from re-throttling to 1.2 GHz; leftovers run after the
            # last real matmul, hidden under the slower engines' drains
            for i in range(48):
                ps = pv_ps.tile([P, NQ], F32, tag="pv", name=f"fill_{i}")
                nc.tensor.matmul(ps[:, 0:NQ // 2], junk_sb[:, 0:P],
                                 junk_sb[:, 0:NQ // 2],
                                 start=True, stop=True)

    nc.compile()
    return nc


def _host_masks():
    k = np.arange(P)[:, None]
    q = np.arange(P)[None, :]
    return (k <= q).astype(np.float32)


def _shard(x, Wq, Wk, Wv, Wout):
    import ml_dtypes
    bf16 = ml_dtypes.bfloat16
    masks = _host_masks()
    in_maps = []
    for c in range(NCORES):
        b, g = c // NH, c % NH
        hs = slice(g * NH, (g + 1) * NH)
        in_maps.append({
            "xT": np.ascontiguousarray(x[b].T).astype(bf16),
            "wq": np.ascontiguousarray(Wq[:, hs, :].reshape(D, EL)).astype(bf16),
            "wk": np.ascontiguousarray(Wk[:, hs, :].reshape(D, EL)).astype(bf16),
            "wv": np.ascontiguousarray(Wv[:, hs, :].reshape(D, EL)).astype(bf16),
            "wout": np.ascontiguousarray(Wout[hs].reshape(EL, D)).astype(bf16),
            "masks": masks.astype(bf16),
        })
    return in_maps


_NC_CACHE = None


def _get_nc():
    global _NC_CACHE
    if _NC_CACHE is None:
        _NC_CACHE = build()
    return _NC_CACHE


def run(x, Wq, Wk, Wv, Wout, trace=False):
    nc = _get_nc()
    in_maps = _shard(np.asarray(x), np.asarray(Wq), np.asarray(Wk),
                     np.asarray(Wv), np.asarray(Wout))
    res = run_bass_kernel_spmd(nc, in_maps, core_ids=list(range(NCORES)),
                               trace=trace)
    parts = [res.results[c]["out"] for c in range(NCORES)]
    full = np.stack([
        parts[0] + parts[1] + parts[2] + parts[3],
        parts[4] + parts[5] + parts[6] + parts[7],
    ]).astype(np.float32)
    return full, res


def kernel(x, Wq, Wk, Wv, Wout):
    for _ in range(3):
        full, _ = run(x, Wq, Wk, Wv, Wout, trace=False)
        if np.isfinite(full).all():
            return full
    return full


# revision 37
# speedup vs baseline: 1.2126x; 1.2126x over previous
"""Causal multi-head attention (B=2, L=2048, D=1024, H=16, Dh=64) on 8 TRN2
NeuronCores.

Sharding: data-parallel over B (2 groups of 4 cores), tensor-parallel over H
within a group (4 heads per core). Each core computes QKV projections for its
heads, full causal attention per head (flash-style, scores kept transposed so
no on-chip transposes are needed), and a partial output projection
y_c = sum_h o_h @ Wout_h. The host sums the 4 partials per batch.

Per-core layout choices:
  - x is pre-transposed on the host (xT [D, L]) so the QKV contraction dim D
    lands on SBUF partitions directly.
  - q, k are produced transposed (qT/kT [e, L]) so the scores matmul
    ST = K_h @ Q_h^T contracts over Dh on partitions; softmax runs on ST
    tiles [k=128, q=512] with the reduction (sum over k) folded into the
    P@V matmul via a ones-row appended to V (lhsT [128, 65]; row 64 of the
    PSUM result is the softmax denominator).
  - Scores/exp/P@V skip columns left of the causal diagonal block.
  - Emission priority: attention steps for a q-chunk are emitted BEFORE the
    filler work (next chunk's QKV, norm-backs, projections) so the Tile
    scheduler always prefers feeding the scalar engine's exp stream and
    fills PE stalls with the later-priority matmuls.
"""

import numpy as np

import concourse.bass as bass
import concourse.mybir as mybir
import concourse.tile as tile
from concourse import bacc
from concourse.bass import broadcast_tensor_aps
from concourse.bass_utils import run_bass_kernel_spmd

F32 = mybir.dt.float32
BF16 = mybir.dt.bfloat16
EXP = mybir.ActivationFunctionType.Exp
MULT = mybir.AluOpType.mult

B, L, D, H = 2, 2048, 1024, 16
Dh = D // H
NCORES = 8
NH = 4            # heads per core
EL = NH * Dh      # local head dims = 256
P = 128
NQ = 512          # q-chunk width (scores free dim)
QC = L // NQ      # 4 q-chunks
DC = D // P       # 8 contraction chunks for projections
LC = 4            # xT l-chunks for QKV
NL = L // LC      # 512


def build():
    nc = bacc.Bacc("TRN2", target_bir_lowering=False, debug=False,
                   num_devices=NCORES)

    xT = nc.dram_tensor("xT", [D, L], BF16, kind="ExternalInput")
    wq = nc.dram_tensor("wq", [D, EL], BF16, kind="ExternalInput")
    wk = nc.dram_tensor("wk", [D, EL], BF16, kind="ExternalInput")
    wv = nc.dram_tensor("wv", [D, EL], BF16, kind="ExternalInput")
    wout = nc.dram_tensor("wout", [EL, D], BF16, kind="ExternalInput")
    masks = nc.dram_tensor("masks", [P, P], BF16, kind="ExternalInput")
    out = nc.dram_tensor("out", [L, D], F32, kind="ExternalOutput")

    scale = 1.0 / np.sqrt(Dh)

    with tile.TileContext(nc) as tc:
        with (
            tc.tile_pool(name="const", bufs=1) as cpool,
            tc.tile_pool(name="xt", bufs=2) as xpool,
            tc.tile_pool(name="pt", bufs=6) as ptpool,
            tc.tile_pool(name="work", bufs=3) as wpool,
            tc.tile_pool(name="norm", bufs=8) as npool,
            tc.tile_pool(name="dram", bufs=8, space="DRAM") as dpool,
            tc.tile_pool(name="mm", bufs=2, space="PSUM") as mm_ps,
            tc.tile_pool(name="st", bufs=2, space="PSUM") as st_ps,
            tc.tile_pool(name="pv", bufs=2, space="PSUM") as pv_ps,
        ):
            # ---- persistent SBUF tensors ----
            wq_sb = cpool.tile([P, DC, EL], BF16, tag="wq")
            wk_sb = cpool.tile([P, DC, EL], BF16, tag="wk")
            wv_sb = cpool.tile([P, DC, EL], BF16, tag="wv")
            wout_sb = cpool.tile([P, EL // P, D], BF16, tag="wout")
            mask_sb = cpool.tile([P, P], BF16, tag="mask")
            qT_sb = cpool.tile([P, EL // P, L], BF16, tag="qT")
            kT_sb = cpool.tile([P, EL // P, L], BF16, tag="kT")
            vext_sb = cpool.tile([P, L // P, NH, Dh + 1], BF16, tag="vext")
            oT_sb = cpool.tile([P, EL // P, L], BF16, tag="oT")
            ones_f32 = cpool.tile([P, P], F32, tag="onesf")
            junk_sb = cpool.tile([P, NQ], BF16, tag="junk")

            # DMA order = need order, with few large dispatches (each
            # dma_start costs ~0.3-0.6us of sequencer dispatch): the first
            # qk matmul group consumes (wq chunk dc, xt0 chunk dc) in dc
            # order, so interleave halves of those first; spread dispatch
            # across the two HWDGE sequencers (SP + ACT, which is idle
            # during the prologue).
            xT_r = xT.ap().rearrange("(o p) l -> p o l", p=P)
            wq_r = wq.ap().rearrange("(o p) e -> p o e", p=P)
            xt0 = xpool.tile([P, DC, NL], BF16, tag="xt", name="xt0")
            hd = DC // 2
            nc.sync.dma_start(wq_sb[:, 0:hd, :], wq_r[:, 0:hd, :])
            nc.scalar.dma_start(xt0[:, 0:hd, :], xT_r[:, 0:hd, 0:NL])
            nc.sync.dma_start(wq_sb[:, hd:, :], wq_r[:, hd:, :])
            nc.scalar.dma_start(xt0[:, hd:, :], xT_r[:, hd:, 0:NL])
            nc.sync.dma_start(mask_sb[:], masks[:, :])
            nc.sync.dma_start(
                wk_sb[:], wk.ap().rearrange("(o p) e -> p o e", p=P))
            nc.scalar.dma_start(
                wv_sb[:], wv.ap().rearrange("(o p) e -> p o e", p=P))
            nc.sync.dma_start(
                wout_sb[:], wout.ap().rearrange("(o p) d -> p o d", p=P))

            nc.vector.memset(junk_sb[:], 0.25)
            nc.vector.memset(ones_f32[:], 1.0)
            # ones column of vext (the softmax-denominator row of P@V)
            nc.vector.tensor_copy(
                out=vext_sb[:, :, :, Dh],
                in_=ones_f32[:, 0:L // P * NH].rearrange("p (a b) -> p a b", a=L // P),
            )
            # preload the exp table set during the DMA wait so the first
            # real exp doesn't pay the ~2.7us table load
            junk2 = wpool.tile([P, Dh], BF16, tag="junk2")
            nc.scalar.activation(out=junk2[:], in_=junk_sb[:, 0:Dh],
                                 func=EXP, scale=1.0)
            # HAM warm-up: dummy matmuls keep the PE busy while the first
            # weight/activation DMAs land, so real work starts at 2.4 GHz
            for i in range(8):
                ps = mm_ps.tile([P, NQ], F32, tag="mm", name=f"warm_{i}")
                nc.tensor.matmul(ps[:], junk_sb[:, 0:P], junk_sb[:],
                                 start=True, stop=True)

            def emit_qkv(lc):
                if lc == 0:
                    xt = xt0
                else:
                    xt = xpool.tile([P, DC, NL], BF16, tag="xt",
                                    name=f"xt{lc}")
                    for dc in range(0, DC, 2):
                        nc.sync.dma_start(
                            xt[:, dc:dc + 2, :],
                            xT_r[:, dc:dc + 2, lc * NL:(lc + 1) * NL])
                # ec-major so attention pair 0 (heads 0/1 = ec 0) unblocks
                # after the first two groups
                gi = 0
                for ec in range(EL // P):
                    for w_sb, dst in ((wq_sb, qT_sb), (wk_sb, kT_sb)):
                        ps = mm_ps.tile([P, NQ], F32, tag="mm",
                                        name=f"qk_{lc}_{ec}")
                        for dc in range(DC):
                            nc.tensor.matmul(
                                ps[:],
                                w_sb[:, dc, ec * P:(ec + 1) * P],
                                xt[:, dc, :],
                                start=(dc == 0), stop=(dc == DC - 1),
                            )
                        nc.vector.tensor_copy(
                            out=dst[:, ec, lc * NL:(lc + 1) * NL], in_=ps[:])
                        if lc == 0:
                            # dependency-free fillers between the first qk
                            # groups: they plug the input-DMA-paced gaps in
                            # the PE stream so the HAM clock gate reaches
                            # 8/8 during the ramp instead of ~20us in
                            for i in range(2):
                                fp = pv_ps.tile([P, NQ], F32, tag="pv",
                                                name=f"rfill_{gi}_{i}")
                                nc.tensor.matmul(
                                    fp[:, 0:NQ // 2], junk_sb[:, 0:P],
                                    junk_sb[:, 0:NQ // 2],
                                    start=True, stop=True)
                            gi += 1

                for lt in range(NL // P):
                    lo = lc * (NL // P) + lt
                    ps = mm_ps.tile([P, EL], F32, tag="mm",
                                    name=f"v_{lc}_{lt}")
                    for dc in range(DC):
                        nc.tensor.matmul(
                            ps[:],
                            xt[:, dc, lt * P:(lt + 1) * P],
                            wv_sb[:, dc, :],
                            start=(dc == 0), stop=(dc == DC - 1),
                        )
                    nc.vector.tensor_copy(
                        out=vext_sb[:, lo, :, 0:Dh],
                        in_=ps[:].rearrange("p (h e) -> p h e", h=NH),
                    )
                    if lc == 0:
                        for i in range(2):
                            fp = pv_ps.tile([P, NQ], F32, tag="pv",
                                            name=f"vfill_{lt}_{i}")
                            nc.tensor.matmul(
                                fp[:, 0:NQ // 2], junk_sb[:, 0:P],
                                junk_sb[:, 0:NQ // 2],
                                start=True, stop=True)

            norm_state = {}        # (qc, h) -> (ot_un, dr2)

            def emit_attn_pair(qc, pair):
                nk = 4 * (qc + 1)          # causal k-chunks of 128
                heads = (2 * pair, 2 * pair + 1)
                pts = {}               # (h, ki) -> pt tile
                pvs = {}               # h -> accumulating PSUM tile

                def emit_pv(ki):
                    # P@V runs one ki behind the scores so the in-order
                    # PE stream never waits on the exp of the current ki
                    cm = max(0, P * (ki - 4 * qc))
                    for h in heads:
                        if ki == 0:
                            pvs[h] = pv_ps.tile([Dh + 1, NQ], F32,
                                                name=f"po_{qc}_{h}",
                                                tag="pv")
                        nc.tensor.matmul(
                            pvs[h][:, cm:],
                            vext_sb[:, ki, h, :],
                            pts.pop((h, ki))[:, cm:],
                            start=(ki == 0), stop=(ki == nk - 1),
                        )

                # scores are emitted in runs of two ki so consecutive score
                # matmuls alternate PE row groups (heads at partitions 0-63
                # / 64-127) and every stationary load after the first hides
                # under the previous matmul's streaming
                for kb in range(0, nk, 2):
                    kis = [ki for ki in (kb, kb + 1) if ki < nk]
                    stps = {}
                    for ki in kis:
                        j = ki - 4 * qc    # >=0 on diagonal-crossing tiles
                        c0 = max(0, P * j)
                        # both heads' score tiles share one 2-bank PSUM
                        # tile so a single EXP covers the pair
                        stp = st_ps.tile([P, 2, NQ], F32, tag="st",
                                         name=f"st_{qc}_{pair}_{ki}")
                        ptp = ptpool.tile([P, 2, NQ], BF16, tag="pt",
                                          name=f"pt_{qc}_{pair}_{ki}")
                        stps[ki] = (stp, ptp, c0, j)
                        for idx, h in enumerate(heads):
                            hp = (h % 2) * 64
                            ec = h // 2
                            nc.tensor.matmul(
                                stp[:, idx, c0:],
                                kT_sb[hp:hp + 64, ec, ki * P:(ki + 1) * P],
                                qT_sb[hp:hp + 64, ec,
                                      qc * NQ + c0:(qc + 1) * NQ],
                                start=True, stop=True,
                            )
                            pts[(h, ki)] = ptp[:, idx, :]
                    for ki in kis:
                        stp, ptp, c0, j = stps[ki]
                        nc.scalar.activation(
                            out=ptp[:, :, c0:], in_=stp[:, :, c0:],
                            func=EXP, scale=scale)
                        if j >= 0:
                            # the diagonal 128-block needs the triangular
                            # mask
                            for idx in range(2):
                                nc.gpsimd.tensor_tensor(
                                    out=ptp[:, idx, c0:c0 + P],
                                    in0=ptp[:, idx, c0:c0 + P],
                                    in1=mask_sb[:, :], op=MULT)
                    # P@V runs one score-group behind so the in-order PE
                    # stream never waits on a fresh exp
                    for ki in (kb - 2, kb - 1):
                        if ki >= 0:
                            emit_pv(ki)
                for ki in (nk - 2, nk - 1):
                    emit_pv(ki)

                # the last q-chunk's chains are the kernel's tail: route
                # their DMA hops through the ACT HWDGE queue (idle by then)
                # so they don't serialize behind the y-output DMA dispatches
                # on the SP sequencer
                deng = nc.scalar if qc == QC - 1 else nc.sync
                for h in heads:
                    po = pvs[h]
                    # norm front half: evacuate PSUM (pinned to DVE so it
                    # never steals exp throughput from the scalar engine)
                    # and kick off the reciprocal chain
                    ot_un = npool.tile([64, NQ], BF16, tag="otun",
                                       name=f"otun_{qc}_{h}")
                    nc.vector.tensor_copy(out=ot_un[:], in_=po[0:64, :])
                    rsum = npool.tile([P, NQ], F32, tag="rsum",
                                      name=f"rsum_{qc}_{h}")
                    nc.vector.tensor_copy(out=rsum[64:65, :],
                                          in_=po[64:65, :])
                    # reshape the [1,512] rsum row to [64,8] with one
                    # SBUF->SBUF DMA so the reciprocal uses 64 DVE lanes
                    r64 = npool.tile([64, NQ // 64], F32, tag="r64",
                                     name=f"r64_{qc}_{h}")
                    deng.dma_start(r64[:], rsum[64:65, :])
                    nc.vector.reciprocal(r64[:], r64[:])
                    dr2 = dpool.tile([NQ], F32,
                                     name=f"dr2_{qc}_{h}", tag="dr2")
                    deng.dma_start(
                        dr2[:].rearrange("(a b) -> a b", b=NQ // 64),
                        r64[:])
                    norm_state[(qc, h)] = (ot_un, dr2)

            def emit_norm_back(qc):
                # odd heads' chains carry an extra partition-shifting DMA
                # hop into oT; start them first on the last q-chunk so the
                # hop overlaps the even heads' in-place multiplies
                order = (1, 3, 0, 2) if qc == QC - 1 else range(NH)
                for h in order:
                    emit_norm_back_h(qc, h)

            def emit_norm_back_h(qc, h):
                # back half: replicate the reciprocal row to 64 partitions
                # with a stride-0 broadcast DMA from the DRAM bounce, then
                # one DVE multiply; no PE involvement
                hp = (h % 2) * 64
                ec = h // 2
                deng = nc.scalar if qc == QC - 1 else nc.sync
                ot_un, dr2 = norm_state.pop((qc, h))
                rr_bc = npool.tile([64, NQ], F32, tag="rrbc",
                                   name=f"rrbc_{qc}_{h}")
                d_ap, s_ap = broadcast_tensor_aps(
                    rr_bc[:, :], dr2[:].rearrange("(a b) -> a b", a=1))
                deng.dma_start(d_ap, s_ap)
                # the last q-chunk's two per-pair multiplies are on the
                # critical tail: run them on different engines in parallel,
                # giving the faster DVE to the odd head whose chain still
                # has the oT DMA hop after the multiply
                meng = nc.gpsimd if (qc == QC - 1 and h % 2 == 0) else nc.vector
                if hp == 0:
                    # heads on partitions 0-63 can write oT in place —
                    # no partition shift needed, so skip the DMA hop
                    meng.tensor_tensor(
                        out=oT_sb[0:64, ec, qc * NQ:(qc + 1) * NQ],
                        in0=ot_un[:], in1=rr_bc[:], op=MULT)
                else:
                    tmp = wpool.tile([64, NQ], BF16, tag="tmp")
                    meng.tensor_tensor(out=tmp[:], in0=ot_un[:],
                                       in1=rr_bc[:], op=MULT)
                    deng.dma_start(
                        oT_sb[hp:hp + 64, ec, qc * NQ:(qc + 1) * NQ],
                        tmp[:])

            def emit_proj(qc):
                # y = oT^T @ wout (partial over heads) for this q-chunk's rows
                for lt in range(4 * qc, 4 * (qc + 1)):
                    emit_proj_lt(lt)

            def emit_proj_lt(lt, pool=None, tag="mm", fine=False):
                for do in range(D // NQ):
                    ps = (pool or mm_ps).tile([P, NQ], F32, tag=tag,
                                              name=f"y_{lt}_{do}")
                    for ec in range(EL // P):
                        nc.tensor.matmul(
                            ps[:],
                            oT_sb[:, ec, lt * P:(lt + 1) * P],
                            wout_sb[:, ec, do * NQ:(do + 1) * NQ],
                            start=(ec == 0), stop=(ec == EL // P - 1),
                        )
                    y_sb = wpool.tile([P, NQ], F32, tag="y")
                    if fine:
                        # tail tiles: split the evacuation + output DMA in
                        # half across both engines/queues so the last
                        # tile's drain chain is as short as possible
                        hn = NQ // 2
                        nc.vector.tensor_copy(out=y_sb[:, 0:hn],
                                              in_=ps[:, 0:hn])
                        nc.scalar.copy(out=y_sb[:, hn:], in_=ps[:, hn:])
                        nc.sync.dma_start(
                            out.ap()[lt * P:(lt + 1) * P,
                                     do * NQ:do * NQ + hn],
                            y_sb[:, 0:hn])
                        nc.scalar.dma_start(
                            out.ap()[lt * P:(lt + 1) * P,
                                     do * NQ + hn:(do + 1) * NQ],
                            y_sb[:, hn:])
                    else:
                        nc.any.tensor_copy(out=y_sb[:], in_=ps[:])
                        nc.sync.dma_start(
                            out.ap()[lt * P:(lt + 1) * P,
                                     do * NQ:(do + 1) * NQ],
                            y_sb[:])

            # attention for q-chunk ph is emitted BEFORE the fillers
            # (QKV ph+1, norm-backs, projections) so the scheduler keeps
            # the exp stream fed and uses fillers only to plug PE stalls
            emit_qkv(0)
            for ph in range(QC):
                emit_attn_pair(ph, 0)
                emit_attn_pair(ph, 1)
                if ph + 1 < QC:
                    emit_qkv(ph + 1)
                if ph < 2:
                    # the first two phases are input-DMA paced; keep the
                    # PE (and with it the HAM clock gate) busy across the
                    # xt-chunk waits
                    for i in range(4):
                        fp = pv_ps.tile([P, NQ], F32, tag="pv",
                                        name=f"pfill_{ph}_{i}")
                        nc.tensor.matmul(
                            fp[:, 0:NQ // 2], junk_sb[:, 0:P],
                            junk_sb[:, 0:NQ // 2],
                            start=True, stop=True)
                if ph >= 1:
                    emit_norm_back(ph - 1)
                if ph >= 2:
                    emit_proj(ph - 2)
            # epilogue: last norm back-halves, then the remaining
            # projections (qc=2 rows are ready; qc=3 rows wait on the
            # final norm-backs)
            emit_norm_back(QC - 1)
            emit_proj(QC - 2)
            for i, lt in enumerate(range(12, 16)):
                if i % 2 == 0:
                    emit_proj_lt(lt, pool=st_ps, tag="st", fine=(lt >= 14))
                else:
                    emit_proj_lt(lt, fine=(lt >= 14))
            # lowest-priority dependency-free matmuls: the scheduler only
            # picks these when nothing real is ready, so they plug PE idle
            # gaps (the tail's norm-chain wait) and keep the HAM clock
            # gate from re-throttling to 1.2 GHz; leftovers run after the
            # last real matmul, hidden under the slower engines' drains
            for i in range(36):
                ps = pv_ps.tile([P, NQ], F32, tag="pv", name=f"fill_{i}")
                nc.tensor.matmul(ps[:, 0:NQ // 2], junk_sb[:, 0:P],
                                 junk_sb[:, 0:NQ // 2],
                                 start=True, stop=True)

    nc.compile()
    return nc


def _host_masks():
    k = np.arange(P)[:, None]
    q = np.arange(P)[None, :]
    return (k <= q).astype(np.float32)


def _shard(x, Wq, Wk, Wv, Wout):
    import ml_dtypes
    bf16 = ml_dtypes.bfloat16
    masks = _host_masks()
    in_maps = []
    for c in range(NCORES):
        b, g = c // NH, c % NH
        hs = slice(g * NH, (g + 1) * NH)
        in_maps.append({
            "xT": np.ascontiguousarray(x[b].T).astype(bf16),
            "wq": np.ascontiguousarray(Wq[:, hs, :].reshape(D, EL)).astype(bf16),
            "wk": np.ascontiguousarray(Wk[:, hs, :].reshape(D, EL)).astype(bf16),
            "wv": np.ascontiguousarray(Wv[:, hs, :].reshape(D, EL)).astype(bf16),
            "wout": np.ascontiguousarray(Wout[hs].reshape(EL, D)).astype(bf16),
            "masks": masks.astype(bf16),
        })
    return in_maps


_NC_CACHE = None


def _get_nc():
    global _NC_CACHE
    if _NC_CACHE is None:
        _NC_CACHE = build()
    return _NC_CACHE


def run(x, Wq, Wk, Wv, Wout, trace=False):
    nc = _get_nc()
    in_maps = _shard(np.asarray(x), np.asarray(Wq), np.asarray(Wk),
                     np.asarray(Wv), np.asarray(Wout))
    res = run_bass_kernel_spmd(nc, in_maps, core_ids=list(range(NCORES)),
                               trace=trace)
    parts = [res.results[c]["out"] for c in range(NCORES)]
    full = np.stack([
        parts[0] + parts[1] + parts[2] + parts[3],
        parts[4] + parts[5] + parts[6] + parts[7],
    ]).astype(np.float32)
    return full, res


def kernel(x, Wq, Wk, Wv, Wout):
    for _ in range(3):
        full, _ = run(x, Wq, Wk, Wv, Wout, trace=False)
        if np.isfinite(full).all():
            return full
    return full
